# revision 67
# baseline (speedup 1.0000x reference)
"""BoundaryDoULoss Trainium2 kernel (v15).

Data-parallel over batch: 16 images sharded 2-per-core across 8 NeuronCores.
Each core computes per-class partial sums (S = region count, C_raw = boundary
ladder count) plus the raw I/Z gram blocks; the host reduces partials,
inverts the ladder, extracts the gram diagonals, and forms alpha + the loss.

Engine split (all ops verified legal on real TRN2 silicon -- GPSIMD has no
compare ops and cannot touch PSUM; the DVE TensorTensor ISA has no divide):
  - DVE: neighbor-diff masks (tensor_tensor not_equal, 2x mode), 4x-mode
    tensor_scalar counts (S via is_equal producing the one-hot planes;
    C ladder via is_ge on yb), the custom-ISA fp32 reciprocal of the
    softmax denominator reading PSUM directly, and the p = e*g multiplies.
  - PE: identity-stationary matmul-adds build the boundary sum
    s = V_up + V_dn + H_l + H_r + 32t in PSUM (values <= 100, exact bf16),
    sum the four exp planes into the softmax denominator, and reduce
    I = sum(p*oh) / Z = sum(p^2) via one PSUM chain per class with a
    256-wide [e|oh] moving operand accumulated across both images.
  - ACT: softmax exps (reading fp8 logits directly), the PSUM->bf16 yb
    conversion (Relu, exact for >= 0), and the I/Z gram drains (Copy);
    all functions live in one activation table.
  - Pool: border memsets, halo sentinels, and the f32->bf16 reciprocal
    downconverts (its throughput is too low for anything bigger).

The schedule software-pipelines per (image, quarter): PE runs masks(img0),
D(q0), D(q1), masks(img1), then alternating IZ(q)/D(q+2) so the
recip->gconv->pmult chain on DVE/Pool for quarter q hides under PE's work
on earlier quarters.  Mask PSUM banks ping-pong 2-wide with the relu
drains interleaved into the ACT exp stream exactly where the bank cycle
needs them.  Head latency is cut by a split target DMA (compares start
under the second half's wire time) and by emitting the halo-independent
mask quarters first; the tail stops the I/Z class chains staggered so the
DVE diag extracts pipeline under the final matmuls.  NOTE: the Tile
framework list-schedules by (readiness, emission priority) — emission
order here is a priority hint, and correctness-critical orderings (the
sentinel memset before the halo DMA, border memsets before the mask
matmuls, the stats DMA after every accum writer) must be enforced by
emission order because overlapping-write hazards follow it.

The logits travel as fp8-e4m3 (quarter of the f32 HBM traffic; per-pixel
quantization noise cancels over the 0.5M-pixel per-class sums). The target
travels as bf16 scaled by 32 (exact) so the DVE compare ops keep their
2-byte fast modes.

The C ladder: yb = 32t + s where s>=1 iff boundary (borders forced via halo
sentinels and H border memsets). raw_c = sum(yb >= 32c+1) counts boundary
pixels of class c plus ALL pixels of classes > c; the host inverts
C_c = raw_c - sum_{c'>c} S_c'.
"""

import numpy as np
import ml_dtypes
import concourse.tile as tile
import concourse.mybir as mybir
from concourse import bacc
from concourse.bass_utils import run_bass_kernel_spmd

N_CORES = 8
B, NCLS, H, W = 16, 4, 512, 512
BL = B // N_CORES  # images per core
R = 4  # rows per partition
P = 128
FW = R * W  # free size of one image tile
HF = FW // 2
QW = FW // 4
SMOOTH = 1e-5
TS = 32.0  # target scale factor (class c encoded as 32c)
SENT = 7.0 * TS  # sentinel (not a class id) for image top/bottom halos

f32 = mybir.dt.float32
bf16 = mybir.dt.bfloat16
f8 = mybir.dt.float8e4
Alu = mybir.AluOpType
AF = mybir.ActivationFunctionType

_cache = {}


def _kernel_body(nc, tc, x_ap, t_ap, idm_ap, out_ap):
    with (
        tc.tile_pool(name="io", bufs=2) as io_pool,
        tc.tile_pool(name="work", bufs=2) as work_pool,
        tc.tile_pool(name="acc", bufs=1) as acc_pool,
        tc.tile_pool(name="ps", bufs=1, space="PSUM") as psum_pool,
    ):
        st = acc_pool.tile([P, 24], f32)
        idm = acc_pool.tile([P, 128], bf16)

        # act-table preload: trigger the 1.3us LoadActFuncSet at t=0 on a
        # dummy tile instead of on the first real exp
        warm = acc_pool.tile([P, 2], bf16)
        nc.vector.memset(warm[:, 0:1], 0.0)
        nc.scalar.activation(warm[:, 1:2], warm[:, 0:1], AF.Exp)

        NCH = FW // 128

        # ---- per-image tiles ------------------------------------------------
        tts, xts, hups, hdns = [], [], [], []
        Vs, Hds, us, ybs, rcps, gs = [], [], [], [], [], []
        for b in range(BL):
            tts.append(io_pool.tile([P, FW], bf16, tag="tt", name=f"tt{b}"))
            xts.append(io_pool.tile([P, NCLS, FW], f8, tag="xt", name=f"xt{b}"))
            hdns.append(io_pool.tile([P, W], bf16, tag="hdn", name=f"hdn{b}"))
            hups.append(io_pool.tile([P, W], bf16, tag="hup", name=f"hup{b}"))
            Vs.append(work_pool.tile([P, 5, W], bf16, tag="V", name=f"V{b}"))
            Hds.append(work_pool.tile([P, FW + 1], bf16, tag="Hd", name=f"Hd{b}"))
            us.append(work_pool.tile([P, 2, NCLS, FW], bf16, tag="u", name=f"u{b}"))
            ybs.append(work_pool.tile([P, FW], bf16, tag="yb", name=f"yb{b}"))
            rcps.append(work_pool.tile([P, FW], f32, tag="rcp32", name=f"rcp{b}"))
            gs.append(work_pool.tile([P, FW], bf16, tag="g", name=f"g{b}"))

        # ---- DMA queue (one hwdge queue, in order) --------------------------
        def dma_image(b):
            tfull = t_ap[b].rearrange("(p r) w -> p (r w)", p=P)
            if b == 0:
                # head-latency critical: idm first (tiny, PE stationary),
                # target in row-pair halves so the DVE compares start under
                # the second half's wire time, then the first logit pair so
                # the exp stream starts early, halos, remaining logits.
                nc.sync.dma_start(idm[:], idm_ap[:])
                nc.sync.dma_start(tts[b][:, 0:HF], tfull[:, 0:HF])
                nc.sync.dma_start(tts[b][:, HF:FW], tfull[:, HF:FW])
                nc.sync.dma_start(
                    xts[b][:, 0:2],
                    x_ap[b, 0:2].rearrange("c (p r) w -> p c (r w)", p=P),
                )
                nc.sync.dma_start(
                    xts[b][:, 2:4],
                    x_ap[b, 2:4].rearrange("c (p r) w -> p c (r w)", p=P),
                )
                nc.sync.dma_start(hdns[b][0 : P - 1, :], t_ap[b, R : H : R, :])
                nc.sync.dma_start(hups[b][1:P, :], t_ap[b, R - 1 : H - 1 : R, :])
            else:
                # logits before halos: img1's exps gate the endgame, while
                # its halo-dependent mask quarters (q0/q3) can run late
                nc.sync.dma_start(tts[b][:], tfull)
                nc.sync.dma_start(
                    xts[b][:, 0:2],
                    x_ap[b, 0:2].rearrange("c (p r) w -> p c (r w)", p=P),
                )
                nc.sync.dma_start(
                    xts[b][:, 2:4],
                    x_ap[b, 2:4].rearrange("c (p r) w -> p c (r w)", p=P),
                )
                nc.sync.dma_start(hdns[b][0 : P - 1, :], t_ap[b, R : H : R, :])
                nc.sync.dma_start(hups[b][1:P, :], t_ap[b, R - 1 : H - 1 : R, :])

        # ---- Pool: sentinels + Hd border memsets + g downconverts -----------
        def pool_sentinels(b):
            # GPSIMD memsets must start at partition 0 (mod 32): fill the
            # whole hdn tile with the sentinel, the halo DMA then overwrites
            # rows 0..126 (the memset lands long before the DMA arrives)
            nc.gpsimd.memset(hdns[b][:, :], SENT)
            nc.gpsimd.memset(hups[b][0:1, :], SENT)

        def pool_hd_borders(b):
            Hd4 = Hds[b][:, 0:FW].rearrange("p (q w) -> p q w", q=R)
            nc.gpsimd.memset(Hd4[:, :, 0:1], 1.0)
            nc.gpsimd.memset(Hds[b][:, FW : FW + 1], 1.0)

        def pool_gconv(b, q):
            qs = slice(q * QW, (q + 1) * QW)
            nc.gpsimd.tensor_scalar(
                gs[b][:, qs], rcps[b][:, qs], 0.0, None, op0=Alu.add
            )

        def act_gconv(b, q):
            # img1's downconverts ride the idle ACT tail (lower latency than
            # the Pool Q7 launch) without displacing any exp
            qs = slice(q * QW, (q + 1) * QW)
            nc.scalar.copy(gs[b][:, qs], rcps[b][:, qs])

        # ---- DVE: masks, one-hots, recips, p-mults, ladders -----------------
        def dve_masks(b):
            tt3 = tts[b][:].rearrange("p (q w) -> p q w", q=R)
            ttf = tts[b][:]
            if b == 0:
                # split by target row-pair halves so the first compares run
                # while the second tt half is still on the wire
                nc.vector.tensor_tensor(
                    Hds[b][:, 1:HF], ttf[:, 0 : HF - 1], ttf[:, 1:HF],
                    op=Alu.not_equal,
                )
                nc.vector.tensor_tensor(
                    Vs[b][:, 1, :], tt3[:, 0, :], tt3[:, 1, :], op=Alu.not_equal
                )
                nc.vector.tensor_tensor(
                    Hds[b][:, HF:FW], ttf[:, HF - 1 : FW - 1], ttf[:, HF:FW],
                    op=Alu.not_equal,
                )
                nc.vector.tensor_tensor(
                    Vs[b][:, 2:4, :], tt3[:, 1:3, :], tt3[:, 2:4, :],
                    op=Alu.not_equal,
                )
            else:
                nc.vector.tensor_tensor(
                    Hds[b][:, 1:FW], ttf[:, 0 : FW - 1], ttf[:, 1:FW],
                    op=Alu.not_equal,
                )
                nc.vector.tensor_tensor(
                    Vs[b][:, 1:4, :], tt3[:, 0:3, :], tt3[:, 1:4, :],
                    op=Alu.not_equal,
                )
            nc.vector.tensor_tensor(
                Vs[b][:, 0, :], tt3[:, 0, :], hups[b][:], op=Alu.not_equal
            )
            nc.vector.tensor_tensor(
                Vs[b][:, 4, :], tt3[:, 3, :], hdns[b][:], op=Alu.not_equal
            )

        def dve_onehot(b):
            ttf = tts[b][:]
            for c in range(NCLS):
                nc.vector.tensor_scalar(
                    us[b][:, 1, c], ttf[:], TS * c, None, op0=Alu.is_equal,
                    op1=Alu.add,
                    accum_out=st[:, NCLS * b + c : NCLS * b + c + 1],
                )

        def dve_recip(b, q, ps_e):
            qs = slice(q * QW, (q + 1) * QW)
            nc.vector.reciprocal_approx_fast(rcps[b][:, qs], ps_e[:])

        def dve_pmult(b, q):
            qs = slice(q * QW, (q + 1) * QW)
            e = us[b][:, 0]
            g_b = gs[b][:, qs].unsqueeze(1).broadcast_to((P, NCLS, QW))
            nc.vector.tensor_tensor(e[:, :, qs], e[:, :, qs], g_b, op=Alu.mult)

        def dve_ladder(b, cls):
            junk = work_pool.tile([P, FW], bf16, tag="junk", name=f"junk{b}{cls[0]}")
            for c in cls:
                nc.vector.tensor_scalar(
                    junk[:], ybs[b][:], TS * c + 1.0, None,
                    op0=Alu.is_ge, op1=Alu.add,
                    accum_out=st[:, 8 + NCLS * b + c : 8 + NCLS * b + c + 1],
                )

        # ---- ACT: exps, mask-psum relu drains, gram drains ------------------
        def act_exp(b, cpair, h):
            hs = slice(h * HF, (h + 1) * HF)
            cs = slice(2 * cpair, 2 * cpair + 2)
            nc.scalar.activation(us[b][:, 0, cs, hs], xts[b][:, cs, hs], AF.Exp)

        def act_relu(b, q, ps_s):
            sl = slice(q * 512, (q + 1) * 512)
            nc.scalar.activation(ybs[b][:, sl], ps_s[:], AF.Relu)

        # ---- PE: mask sums, denominators, I/Z gram chains -------------------
        mask_ps = {}

        def pe_mask(b, q, bank):
            Vf = Vs[b][:].rearrange("p q w -> p (q w)")
            Hd = Hds[b]
            ttf = tts[b][:]
            # tt first: it's the earliest-ready mover, so the in-order PE
            # starts each chain (and its p-state ramp) sooner
            movers = [
                ttf, Vf[:, 0:FW], Vf[:, W : W + FW], Hd[:, 0:FW], Hd[:, 1 : FW + 1]
            ]
            sl = slice(q * 512, (q + 1) * 512)
            ps_s = psum_pool.tile([P, 512], f32, tag=bank, name=f"ps_s{b}{q}")
            for mi, mv in enumerate(movers):
                nc.tensor.matmul(
                    ps_s[:], idm[:], mv[:, sl],
                    start=(mi == 0), stop=(mi == len(movers) - 1),
                )
            mask_ps[(b, q)] = ps_s

        den_ps = {}

        def pe_den(b, q, bank):
            qs = slice(q * QW, (q + 1) * QW)
            e = us[b][:, 0]
            ps_e = psum_pool.tile([P, 512], f32, tag=bank, name=f"ps_se{b}{q}")
            for c in range(NCLS):
                nc.tensor.matmul(
                    ps_e[:], idm[:], e[:, c, qs],
                    start=(c == 0), stop=(c == NCLS - 1),
                )
            den_ps[(b, q)] = ps_e

        ps_iz = [
            psum_pool.tile([P, 512], f32, tag=f"iz{c}", name=f"ps_iz{c}")
            for c in range(NCLS)
        ]

        def pe_iz(b, q):
            # 4 chunks of quarter q for each class; one accumulation chain
            # per class spans both images.  On the final quarter the chains
            # stop class-by-class so the ACT gram drains + DMAs pipeline
            # under the remaining matmuls.
            e = us[b][:, 0]
            for c in range(NCLS):
                for ch in range(4):
                    k = q * 4 + ch
                    sl = slice(k * 128, (k + 1) * 128)
                    out2 = ps_iz[c][:, 0:256].rearrange("p (v w) -> p v w", v=2)
                    nc.tensor.matmul(
                        out2, e[:, c, sl], us[b][:, :, c, sl],
                        start=(b == 0 and q == 0 and ch == 0),
                        stop=(b == BL - 1 and q == 3 and ch == 3),
                    )
                if b == BL - 1 and q == 3:
                    # diagonal extracts on DVE (idle at the tail; GPSIMD
                    # cannot access PSUM) straight into the stats tile so
                    # only one tiny DMA rides the tail
                    j128z = work_pool.tile([P, 128], f32, tag=f"j128z{c}")
                    nc.vector.scalar_tensor_tensor(
                        out=j128z[:], in0=ps_iz[c][:, 0:128], scalar=0.0,
                        in1=idm[:], op0=Alu.bypass, op1=Alu.mult,
                        accum_out=st[:, 20 + c : 20 + c + 1],
                    )
                    j128i = work_pool.tile([P, 128], f32, tag=f"j128i{c}")
                    nc.vector.scalar_tensor_tensor(
                        out=j128i[:], in0=ps_iz[c][:, 128:256], scalar=0.0,
                        in1=idm[:], op0=Alu.bypass, op1=Alu.mult,
                        accum_out=st[:, 16 + c : 16 + c + 1],
                    )

        # =====================================================================
        # Emission script.  Per-engine program order == emission order; the
        # interleave below is the software pipeline described in the header.
        # =====================================================================
        # sentinels first: the full-tile hdn memset overlaps the halo DMA
        # rows, and WAW ordering follows emission order
        pool_sentinels(0)
        pool_sentinels(1)
        dma_image(0)

        dve_masks(0)

        pool_hd_borders(0)
        # q1/q2 first: they depend only on the interior V rows, so PE can
        # start ~1us before the halo-dependent q0/q3 are ready
        for q, bank in ((1, "s0"), (2, "s1"), (0, "s0"), (3, "s1")):
            pe_mask(0, q, bank)
        dve_onehot(0)

        # ACT: img0 exps with the mask-psum relu drains interleaved exactly
        # where the 2-bank ping-pong needs them free (bank order follows the
        # q1,q2,q0,q3 mask emission)
        act_exp(0, 0, 0)
        act_relu(0, 1, mask_ps[(0, 1)])
        act_exp(0, 1, 0)
        act_relu(0, 2, mask_ps[(0, 2)])
        act_relu(0, 0, mask_ps[(0, 0)])
        act_relu(0, 3, mask_ps[(0, 3)])

        dma_image(1)
        dve_masks(1)
        pool_hd_borders(1)

        pe_den(0, 0, "se0")
        pe_den(0, 1, "se1")

        for q, bank in ((1, "s0"), (2, "s1")):
            pe_mask(1, q, bank)
        act_exp(0, 0, 1)
        act_relu(1, 1, mask_ps[(1, 1)])
        act_relu(1, 2, mask_ps[(1, 2)])
        for q, bank in ((0, "s0"), (3, "s1")):
            pe_mask(1, q, bank)
        act_exp(0, 1, 1)

        dve_recip(0, 0, den_ps[(0, 0)])
        pool_gconv(0, 0)
        dve_pmult(0, 0)
        dve_recip(0, 1, den_ps[(0, 1)])
        pool_gconv(0, 1)
        dve_pmult(0, 1)
        dve_onehot(1)

        # img1 exps ahead of the remaining mask drains (those gate nothing
        # but the img1 ladder, which runs late anyway)
        act_exp(1, 0, 0)
        act_exp(1, 1, 0)
        act_exp(1, 0, 1)
        act_exp(1, 1, 1)
        act_relu(1, 0, mask_ps[(1, 0)])
        act_relu(1, 3, mask_ps[(1, 3)])

        pe_iz(0, 0)
        pe_den(0, 2, "se0")
        dve_recip(0, 2, den_ps[(0, 2)])
        pool_gconv(0, 2)
        dve_pmult(0, 2)

        pe_iz(0, 1)
        pe_den(0, 3, "se1")
        dve_recip(0, 3, den_ps[(0, 3)])
        dve_ladder(0, [0, 1])
        pool_gconv(0, 3)
        dve_pmult(0, 3)
        dve_ladder(0, [2, 3])

        pe_iz(0, 2)
        # img1 denominators start in the freed mask banks so two chains can
        # be in flight while img0's gram chunks still stream
        pe_den(1, 0, "s0")
        dve_recip(1, 0, den_ps[(1, 0)])
        dve_ladder(1, [0])
        act_gconv(1, 0)
        dve_pmult(1, 0)

        pe_den(1, 1, "s1")
        pe_iz(0, 3)
        dve_recip(1, 1, den_ps[(1, 1)])
        dve_ladder(1, [1])
        act_gconv(1, 1)
        dve_pmult(1, 1)

        pe_iz(1, 0)
        pe_den(1, 2, "se0")
        dve_recip(1, 2, den_ps[(1, 2)])
        act_gconv(1, 2)
        dve_pmult(1, 2)

        pe_iz(1, 1)
        pe_den(1, 3, "se1")
        dve_recip(1, 3, den_ps[(1, 3)])
        # one ladder fills the gconv round-trip exactly; the other runs
        # under the final gram chunks so pmult1q3 isn't pushed out
        dve_ladder(1, [2])
        act_gconv(1, 3)
        dve_pmult(1, 3)
        dve_ladder(1, [3])

        pe_iz(1, 2)
        pe_iz(1, 3)
        nc.sync.dma_start(out_ap[:], st[:])


def _build():
    if "nc" in _cache:
        return _cache["nc"]
    nc = bacc.Bacc("TRN2", target_bir_lowering=False, debug=False, num_devices=N_CORES)
    x_ap = nc.dram_tensor("x", [BL, NCLS, H, W], f8, kind="ExternalInput").ap()
    t_ap = nc.dram_tensor("t", [BL, H, W], bf16, kind="ExternalInput").ap()
    idm_ap = nc.dram_tensor("idm", [P, 128], bf16, kind="ExternalInput").ap()
    out_ap = nc.dram_tensor("stats", [P, 24], f32, kind="ExternalOutput").ap()
    with tile.TileContext(nc) as tc:
        _kernel_body(nc, tc, x_ap, t_ap, idm_ap, out_ap)
    nc.compile()
    _cache["nc"] = nc
    return nc


def _finish(stats_sum):
    """stats_sum: [16] = [S[4], C_raw[4], I[4], Z[4]] -> scalar loss."""
    s = stats_sum.astype(np.float64)
    S, C_raw, I, Z = s[0:4], s[4:8], s[8:12], s[12:16]
    # invert the ladder: raw_c = C_c + sum_{c'>c} S_c'
    C = np.zeros(4)
    for c in range(4):
        C[c] = C_raw[c] - S[c + 1 :].sum()
    alpha = 1.0 - (C + SMOOTH) / (S + SMOOTH)
    alpha = np.minimum(2.0 * alpha - 1.0, 0.8)
    loss_c = (Z + S - 2.0 * I + SMOOTH) / (Z + S - (1.0 + alpha) * I + SMOOTH)
    return np.float32(loss_c.mean())


def kernel(inputs: np.ndarray, target: np.ndarray) -> np.ndarray:
    nc = _build()
    x = np.ascontiguousarray(inputs.astype(ml_dtypes.float8_e4m3fn))
    t = np.ascontiguousarray((target.astype(np.float32) * TS).astype(ml_dtypes.bfloat16))
    idm = np.eye(P, dtype=ml_dtypes.bfloat16)
    in_maps = [
        {"x": x[c * BL : (c + 1) * BL], "t": t[c * BL : (c + 1) * BL], "idm": idm}
        for c in range(N_CORES)
    ]
    for attempt in range(3):
        res = run_bass_kernel_spmd(nc, in_maps, list(range(N_CORES)))
        stats = np.zeros(16, dtype=np.float64)
        for c in range(N_CORES):
            s = res.results[c]["stats"].astype(np.float64).sum(axis=0)
            g = np.concatenate([
                s[0:8].reshape(2, NCLS).sum(axis=0),   # S
                s[8:16].reshape(2, NCLS).sum(axis=0),  # C_raw
                s[16:20],                               # I
                s[20:24],                               # Z
            ])
            stats += g
        # S counts must equal the pixel total; retry on transient device faults
        if np.isfinite(stats).all() and abs(stats[0:4].sum() - B * H * W) < 0.5:
            break
    return _finish(stats)
